# revision 1
# baseline (speedup 1.0000x reference)
"""Trainium2 Bass kernel for nn_MultiModalInputEmbeddings.

Data-parallel over batch: 8 cores x 8 batch rows = 4096 tokens/core.
Token slot convention is column-major: token t <-> (partition t%128, slot
t//128), matching the dma_gather/dma_scatter_add custom-op layout.

Per core:
  - Branch logic folds into one bf16 "combined table" gather:
      ctab[0:1000]   = prop_emb + type_emb[0]     (word tokens)
      ctab[1000:1003]= type_emb[3:6]              (special tokens)
      ctab[1003]     = 0                          (smiles placeholder)
      ctab[1004]     = val_b + type_emb[2]        (value tokens)
  - Dense pass (all tokens): e = ctab-row + pos-row + mval*val_w, built by
    accumulating bf16 identity-matmuls into fp32 PSUM (no DVE adds), then
    LayerNorm straight off PSUM (bn_stats/bn_aggr + one tensor_scalar).
    Smiles rows are zeroed for free by folding (1-m_smiles) into the LN
    scale, and the whole pass is written with one plain strided DMA per
    group.
  - SMILES tokens are stream-compacted on device (cross-partition prefix
    sum via triangular matmul + per-row Hillis-Steele + indirect scatter of
    packed (token,pos) records), their fingerprints gathered compactly,
    run through the 768->3072->768 FFN in bf16 (fp32 PSUM accumulation),
    transposed back token-major into PSUM where pos rows join via identity
    matmuls, LayerNorm'd, and dma_scatter_add'ed onto the (zeroed) output
    rows. Compaction padding routes to dump rows past the real output.
"""

import sys

try:
    import concourse  # noqa: F401
except ImportError:  # pragma: no cover
    sys.path.insert(0, "/opt/trn_rl_repo")

import numpy as np

import concourse.bacc as bacc
import concourse.bass as bass  # noqa: F401
import concourse.mybir as mybir
import concourse.tile as tile
from concourse import bass_utils
from concourse.bass import IndirectOffsetOnAxis

F32 = mybir.dt.float32
BF16 = mybir.dt.bfloat16
I32 = mybir.dt.int32
I16 = mybir.dt.int16
ALU = mybir.AluOpType
ACTF = mybir.ActivationFunctionType

B, S, FP, HID = 64, 512, 768, 768
N_CORES = 8
B_LOC = B // N_CORES
N_TOK = B_LOC * S            # 4096 tokens/core
KJ = N_TOK // 128            # 32 slots per partition
NW = N_TOK // 16             # 256 wrapped-index columns
COL_VOCAB, MAX_POS = 1000, 512
H4 = 4 * FP
CTAB_ROWS = COL_VOCAB + 5
ZROW = COL_VOCAB + 3
VROW = COL_VOCAB + 4
DUMP = N_TOK                 # output dump row for compaction padding

S_BLKS = (512, 384)          # smiles capacity 896 = mean 683 + 8.9 sigma
CAP_S = sum(S_BLKS)
DG = 4                       # dense token-tiles per group
EPS = 1e-12


def _replicated_load(nc, dst, src_ap):
    """Load a [16, C] DRAM view into all 8 GPSIMD 16-partition groups."""
    for k in range(8):
        nc.sync.dma_start(out=dst[16 * k : 16 * k + 16, :], in_=src_ap)


def build_program(skip_gb: bool):
    nc = bacc.Bacc(
        "TRN2",
        target_bir_lowering=False,
        debug=False,
        enable_asserts=False,
        num_devices=N_CORES,
    )

    def din(name, shape, dt=F32):
        return nc.dram_tensor(name, shape, dt, kind="ExternalInput").ap()

    fps = din("fps", [N_TOK, FP])
    wtok = din("wtok", [N_TOK], I32)
    vals = din("vals", [N_TOK])
    ttyp = din("ttyp", [N_TOK], I32)
    posi = din("posi", [N_TOK], I32)
    fc1_w = din("fc1_w", [FP, H4])
    fc1_b = din("fc1_b", [H4])
    fc2_w = din("fc2_w", [H4, HID])
    fc2_b = din("fc2_b", [HID])
    prop = din("prop", [COL_VOCAB, HID])
    val_w = din("val_w", [HID])
    val_b = din("val_b", [HID])
    pose = din("pose", [MAX_POS, HID])
    typee = din("typee", [8, HID])
    ln_g = din("ln_g", [HID])
    ln_b = din("ln_b", [HID])
    ident_d = din("ident", [128, 128])
    identbf_d = din("identbf", [128, 128], BF16)
    lexclt_d = din("lexclt", [128, 128])
    ones_col_d = din("ones_col", [128, 1])
    ones_row_d = din("ones_row", [1, 128])
    iota_c_d = din("iota_c", [128, KJ], I32)

    out = nc.dram_tensor("out", [N_TOK + 128, HID], F32, kind="ExternalOutput").ap()
    ctab = nc.dram_tensor("ctab", [CTAB_ROWS, HID], F32, kind="Internal").ap()
    packed = nc.dram_tensor("packed", [1024, 2], I32, kind="Internal").ap()

    from contextlib import ExitStack

    with tile.TileContext(nc) as tc, ExitStack() as es:
        cpool = es.enter_context(tc.tile_pool(name="const", bufs=1))
        wpool = es.enter_context(tc.tile_pool(name="wts", bufs=1))
        spool = es.enter_context(tc.tile_pool(name="small", bufs=1))
        epool = es.enter_context(tc.tile_pool(name="emb", bufs=2))
        fpool = es.enter_context(tc.tile_pool(name="ffn", bufs=1))
        ppool = es.enter_context(tc.tile_pool(name="psum", bufs=1, space="PSUM"))

        # ---- constants ----
        ident = cpool.tile([128, 128], F32)
        nc.sync.dma_start(out=ident[:], in_=ident_d[:])
        identbf = cpool.tile([128, 128], BF16)
        nc.sync.dma_start(out=identbf[:], in_=identbf_d[:])
        lexclt = cpool.tile([128, 128], F32)
        nc.sync.dma_start(out=lexclt[:], in_=lexclt_d[:])
        ones_col = cpool.tile([128, 1], F32)
        nc.sync.dma_start(out=ones_col[:], in_=ones_col_d[:])
        ones_row = cpool.tile([1, 128], F32)
        nc.sync.dma_start(out=ones_row[:], in_=ones_row_d[:])
        iota_c = cpool.tile([128, KJ], I32)
        nc.sync.dma_start(out=iota_c[:], in_=iota_c_d[:])
        eps_t = cpool.tile([128, 1], F32)
        nc.vector.memset(eps_t[:], EPS)

        # ---- weights (bf16 via SWDGE cast-load) ----
        w1 = wpool.tile([128, FP // 128, H4], BF16)
        nc.gpsimd.dma_start(out=w1[:], in_=fc1_w.rearrange("(k p) m -> p k m", p=128))
        w2 = wpool.tile([128, H4 // 128, HID], BF16)
        nc.gpsimd.dma_start(out=w2[:], in_=fc2_w.rearrange("(k p) m -> p k m", p=128))
        b1 = cpool.tile([128, H4 // 128], F32)
        nc.sync.dma_start(out=b1[:], in_=fc1_b.rearrange("(m p) -> p m", p=128))
        b2 = cpool.tile([128, HID // 128], F32)
        nc.sync.dma_start(out=b2[:], in_=fc2_b.rearrange("(m p) -> p m", p=128))
        t1pm = spool.tile([128, HID // 128], F32, tag="t1pm")
        nc.sync.dma_start(out=t1pm[:], in_=typee[1, :].rearrange("(m p) -> p m", p=128))
        nc.vector.tensor_tensor(out=b2[:], in0=b2[:], in1=t1pm[:], op=ALU.add)

        t0b = cpool.tile([128, HID], F32)
        nc.sync.dma_start(out=t0b[:], in_=typee[0:1, :].to_broadcast([128, HID]))
        vwb = cpool.tile([128, HID], F32)
        nc.sync.dma_start(out=vwb[:], in_=val_w[None, :].to_broadcast([128, HID]))
        vbrow = cpool.tile([1, HID], F32)
        nc.sync.dma_start(out=vbrow[:], in_=val_b[None, :])
        t2row = spool.tile([1, HID], F32, tag="t2row")
        nc.sync.dma_start(out=t2row[:], in_=typee[2:3, :])
        nc.vector.tensor_tensor(out=vbrow[:], in0=vbrow[:], in1=t2row[:], op=ALU.add)

        if not skip_gb:
            gb = cpool.tile([128, HID], F32)
            nc.sync.dma_start(out=gb[:], in_=ln_g[None, :].to_broadcast([128, HID]))
            bb = cpool.tile([128, HID], F32)
            nc.sync.dma_start(out=bb[:], in_=ln_b[None, :].to_broadcast([128, HID]))

        # ---- build bf16 ctab / posebf in DRAM ----
        for c in range((COL_VOCAB + 127) // 128):
            r0, r1 = c * 128, min(c * 128 + 128, COL_VOCAB)
            nrow = r1 - r0
            ch = spool.tile([128, HID], F32, tag="ctab_ch")
            nc.sync.dma_start(out=ch[:nrow], in_=prop[r0:r1, :])
            nc.vector.tensor_tensor(out=ch[:nrow], in0=ch[:nrow], in1=t0b[:nrow], op=ALU.add)
            nc.sync.dma_start(out=ctab[r0:r1, :], in_=ch[:nrow])
        chsp = spool.tile([5, HID], F32, tag="chsp")
        nc.vector.memset(chsp[:], 0.0)
        nc.sync.dma_start(out=chsp[0:3, :], in_=typee[3:6, :])
        nc.sync.dma_start(out=chsp[4:5, :], in_=vbrow[:])
        nc.sync.dma_start(out=ctab[COL_VOCAB:CTAB_ROWS, :], in_=chsp[:])

        # ---- wrapped (replicated x8) index tiles ----
        tt_w = cpool.tile([128, NW], I32)
        _replicated_load(nc, tt_w, ttyp.rearrange("(s p) -> p s", p=16))
        wt_w = spool.tile([128, NW], I32, tag="wt_w")
        _replicated_load(nc, wt_w, wtok.rearrange("(s p) -> p s", p=16))
        pos_w32 = spool.tile([128, NW], I32, tag="pos_w32")
        _replicated_load(nc, pos_w32, posi.rearrange("(s p) -> p s", p=16))
        pos16 = cpool.tile([128, NW], I16)
        nc.vector.tensor_copy(pos16[:], pos_w32[:])

        m_w_w = spool.tile([128, NW], I32, tag="m_w_w")
        nc.vector.tensor_scalar(m_w_w[:], tt_w[:], 0, None, ALU.is_equal)
        m_sp_w = spool.tile([128, NW], I32, tag="m_sp_w")
        nc.vector.tensor_scalar(m_sp_w[:], tt_w[:], 3, None, ALU.is_ge)
        m_v_w = spool.tile([128, NW], I32, tag="m_v_w")
        nc.vector.tensor_scalar(m_v_w[:], tt_w[:], 2, None, ALU.is_equal)
        cidx32 = spool.tile([128, NW], I32, tag="cidx32")
        nc.vector.memset(cidx32[:], ZROW)
        nc.vector.copy_predicated(cidx32[:], m_w_w[:], wt_w[:])
        tspec = spool.tile([128, NW], I32, tag="tspec")
        nc.vector.tensor_scalar(tspec[:], tt_w[:], COL_VOCAB - 3, None, ALU.add)
        nc.vector.copy_predicated(cidx32[:], m_sp_w[:], tspec[:])
        vrow_w = spool.tile([128, NW], I32, tag="vrow_w")
        nc.vector.memset(vrow_w[:], VROW)
        nc.vector.copy_predicated(cidx32[:], m_v_w[:], vrow_w[:])
        cidx16 = cpool.tile([128, NW], I16)
        nc.vector.tensor_copy(cidx16[:], cidx32[:])

        # ---- column-major per-token tiles ----
        tt_c = cpool.tile([128, KJ], I32)
        nc.sync.dma_start(out=tt_c[:], in_=ttyp.rearrange("(j p) -> p j", p=128))
        va_c = spool.tile([128, KJ], F32, tag="va_c")
        nc.sync.dma_start(out=va_c[:], in_=vals.rearrange("(j p) -> p j", p=128))
        pos_c = spool.tile([128, KJ], I32, tag="pos_c")
        nc.sync.dma_start(out=pos_c[:], in_=posi.rearrange("(j p) -> p j", p=128))

        m_s_ci = cpool.tile([128, KJ], I32)
        nc.vector.tensor_scalar(m_s_ci[:], tt_c[:], 1, None, ALU.is_equal)
        m_ns_ci = spool.tile([128, KJ], I32, tag="m_ns_ci")
        nc.vector.tensor_scalar(m_ns_ci[:], tt_c[:], 1, None, ALU.not_equal)
        m_s_c = cpool.tile([128, KJ], F32)
        nc.vector.tensor_copy(m_s_c[:], m_s_ci[:])
        notm_c = cpool.tile([128, KJ], F32)
        nc.vector.tensor_scalar(notm_c[:], m_s_c[:], -1.0, 1.0, ALU.mult, ALU.add)
        m_v_ci = spool.tile([128, KJ], I32, tag="m_v_ci")
        nc.vector.tensor_scalar(m_v_ci[:], tt_c[:], 2, None, ALU.is_equal)
        m_v_cf = spool.tile([128, KJ], F32, tag="m_v_cf")
        nc.vector.tensor_copy(m_v_cf[:], m_v_ci[:])
        mval_c = cpool.tile([128, KJ], F32)
        nc.vector.tensor_tensor(out=mval_c[:], in0=va_c[:], in1=m_v_cf[:], op=ALU.mult)

        # ---- smiles compaction ----
        exc_ps = ppool.tile([128, KJ], F32, tag="tp")
        nc.tensor.matmul(out=exc_ps[:], lhsT=lexclt[:], rhs=m_s_c[:])
        exc = spool.tile([128, KJ], F32, tag="exc")
        nc.vector.tensor_copy(exc[:], exc_ps[:])
        cs_ps = ppool.tile([1, KJ], F32, tag="tp")
        nc.tensor.matmul(out=cs_ps[:], lhsT=ones_col[:], rhs=m_s_c[:])
        csum = spool.tile([1, KJ], F32, tag="csum")
        nc.vector.tensor_copy(csum[:], cs_ps[:])
        cur = csum
        for sh in (1, 2, 4, 8, 16):
            nxt = spool.tile([1, KJ], F32, tag=f"cs{sh}")
            nc.vector.tensor_copy(nxt[:], cur[:])
            nc.vector.tensor_tensor(
                out=nxt[:, sh:], in0=cur[:, sh:], in1=cur[:, : KJ - sh], op=ALU.add
            )
            cur = nxt
        base_row = spool.tile([1, KJ], F32, tag="base_row")
        nc.vector.tensor_tensor(out=base_row[:], in0=cur[:], in1=csum[:], op=ALU.subtract)
        bb_ps = ppool.tile([128, KJ], F32, tag="tp")
        nc.tensor.matmul(out=bb_ps[:], lhsT=ones_row[:], rhs=base_row[:])
        slot_f = spool.tile([128, KJ], F32, tag="slot_f")
        nc.vector.tensor_tensor(out=slot_f[:], in0=exc[:], in1=bb_ps[:], op=ALU.add)
        caps_t = spool.tile([128, KJ], F32, tag="caps_t")
        nc.vector.memset(caps_t[:], float(CAP_S))
        nc.vector.copy_predicated(slot_f[:], m_ns_ci[:], caps_t[:])
        slot_i = spool.tile([128, KJ], I32, tag="slot_i")
        nc.vector.tensor_copy(slot_i[:], slot_f[:])

        pack_c = spool.tile([128, KJ, 2], I32, tag="pack_c")
        nc.vector.tensor_copy(pack_c[:, :, 0], iota_c[:])
        nc.vector.tensor_copy(pack_c[:, :, 1], pos_c[:])
        pinit = spool.tile([128, 8, 2], I32, tag="pinit")
        nc.vector.memset(pinit[:], 0)
        nc.vector.memset(pinit[:, :, 0:1], DUMP)
        nc.sync.dma_start(out=packed.rearrange("(p j) t -> p j t", p=128), in_=pinit[:])
        for j in range(KJ):
            nc.gpsimd.indirect_dma_start(
                out=packed[:],
                out_offset=IndirectOffsetOnAxis(ap=slot_i[:, j : j + 1], axis=0),
                in_=pack_c[:, j, :],
                in_offset=None,
            )
        slist32 = spool.tile([128, CAP_S // 16], I32, tag="slist32")
        _replicated_load(
            nc, slist32, packed[:CAP_S, 0].rearrange("(s p) -> p s", p=16)
        )
        psm32 = spool.tile([128, CAP_S // 16], I32, tag="psm32")
        _replicated_load(
            nc, psm32, packed[:CAP_S, 1].rearrange("(s p) -> p s", p=16)
        )
        dest16 = cpool.tile([128, CAP_S // 16], I16)
        nc.vector.tensor_copy(dest16[:], slist32[:])
        sfps16 = cpool.tile([128, CAP_S // 16], I16)
        nc.vector.tensor_scalar(sfps16[:], slist32[:], N_TOK - 1, None, ALU.min)
        psm16 = cpool.tile([128, CAP_S // 16], I16)
        nc.vector.tensor_copy(psm16[:], psm32[:])

        def ln_apply(x512, x256, o768, rs, nb, zero_col=None):
            """LayerNorm apply from two PSUM halves into an SBUF f32 tile."""
            st = spool.tile([128, 2, 6], F32, tag="ln_st", bufs=3)
            mv = spool.tile([128, 2], F32, tag="ln_mv", bufs=3)
            nc.vector.bn_stats(st[:, 0, :], x512)
            nc.vector.bn_stats(st[:, 1, :], x256)
            nc.vector.bn_aggr(mv[:], st[:])
            std = spool.tile([128, 1], F32, tag="ln_std", bufs=3)
            nc.scalar.activation(std[:], mv[:, 1:2], ACTF.Sqrt, bias=eps_t[:, 0:1], scale=1.0)
            nc.vector.reciprocal(rs[:], std[:])
            if zero_col is not None and skip_gb:
                nc.vector.tensor_tensor(out=rs[:], in0=rs[:], in1=zero_col, op=ALU.mult)
            nc.vector.tensor_scalar(nb[:], mv[:, 0:1], rs[:, 0:1], -1.0, ALU.mult, ALU.mult)
            nc.vector.tensor_scalar(o768[:, 0:512], x512, rs[:, 0:1], nb[:, 0:1], ALU.mult, ALU.add)
            nc.vector.tensor_scalar(o768[:, 512:768], x256, rs[:, 0:1], nb[:, 0:1], ALU.mult, ALU.add)

        def gb_apply(o768, zero_col=None):
            if skip_gb:
                return
            nc.vector.tensor_tensor(out=o768[:], in0=o768[:], in1=gb[:], op=ALU.mult)
            nc.vector.tensor_tensor(out=o768[:], in0=o768[:], in1=bb[:], op=ALU.add)
            if zero_col is not None:
                nc.vector.tensor_scalar(o768[:], o768[:], zero_col, None, ALU.mult)

        # ---- dense pass ----
        for g in range(KJ // DG):
            j0 = g * DG
            wcols = slice(j0 * 8, (j0 + DG) * 8)
            cgt = epool.tile([128, DG, HID], F32, tag="C", bufs=1)
            nc.gpsimd.dma_gather(
                cgt[:],
                ctab[:], cidx16[:, wcols], DG * 128, DG * 128, HID,
            )
            pgt = epool.tile([128, DG, HID], F32, tag="P", bufs=1)
            nc.gpsimd.dma_gather(
                pgt[:],
                pose[:], pos16[:, wcols], DG * 128, DG * 128, HID,
            )
            og = epool.tile([128, DG, HID], F32, tag="O", bufs=1)
            for jj in range(DG):
                j = j0 + jj
                vt = spool.tile([128, HID], F32, tag="vtmp", bufs=2)
                nc.scalar.activation(
                    vt[:], vwb[:], ACTF.Copy, bias=0.0, scale=mval_c[:, j : j + 1]
                )
                e_ps = ppool.tile([128, HID], F32, tag="smps", bufs=2)
                for lo, hi in ((0, 512), (512, 768)):
                    nc.tensor.matmul(
                        out=e_ps[:, lo:hi], lhsT=ident[:], rhs=cgt[:, jj, lo:hi],
                        start=True, stop=False, skip_group_check=True,
                    )
                    nc.tensor.matmul(
                        out=e_ps[:, lo:hi], lhsT=ident[:], rhs=pgt[:, jj, lo:hi],
                        start=False, stop=False, skip_group_check=True,
                    )
                    nc.tensor.matmul(
                        out=e_ps[:, lo:hi], lhsT=ident[:], rhs=vt[:, lo:hi],
                        start=False, stop=True, skip_group_check=True,
                    )
                rs = spool.tile([128, 1], F32, tag="ln_rs", bufs=3)
                nbt = spool.tile([128, 1], F32, tag="ln_nb", bufs=3)
                ln_apply(
                    e_ps[:, 0:512], e_ps[:, 512:768], og[:, jj, :], rs, nbt,
                    zero_col=notm_c[:, j : j + 1],
                )
                gb_apply(og[:, jj, :], zero_col=notm_c[:, j : j + 1])
            nc.sync.dma_start(
                out=out[:N_TOK, :].rearrange("(j p) f -> p j f", p=128)[:, j0 : j0 + DG, :],
                in_=og[:],
            )

        # ---- SMILES FFN ----
        joff = 0
        for blk, nb_tok in enumerate(S_BLKS):
            kb = nb_tok // 128
            wcols = slice(joff // 16, (joff + nb_tok) // 16)
            xg = fpool.tile([128, 4, FP], F32, tag="xtok")
            nc.gpsimd.dma_gather(
                xg[:, :kb, :],
                fps[:], sfps16[:, wcols], nb_tok, nb_tok, FP,
            )
            xfm = fpool.tile([128, FP // 128, 512], BF16, tag="xfm")
            for ct in range(kb):
                for k in range(FP // 128):
                    tp = ppool.tile([128, 128], F32, tag="tp")
                    nc.tensor.transpose(
                        out=tp[:], in_=xg[:, ct, k * 128 : (k + 1) * 128], identity=ident[:]
                    )
                    dst = xfm[:, k, ct * 128 : (ct + 1) * 128]
                    if (ct * 6 + k) % 2 == 0:
                        nc.vector.tensor_copy(dst, tp[:])
                    else:
                        nc.scalar.copy(dst, tp[:])

            hid = fpool.tile([128, H4 // 128, 512], BF16, tag="hid")
            for m in range(H4 // 128):
                ph = ppool.tile([128, 512], F32, tag="mm", bufs=2)
                for k in range(FP // 128):
                    nc.tensor.matmul(
                        out=ph[:, :nb_tok],
                        lhsT=w1[:, k, m * 128 : (m + 1) * 128],
                        rhs=xfm[:, k, :nb_tok],
                        start=(k == 0),
                        stop=(k == FP // 128 - 1),
                    )
                if m % 2 == 0:
                    nc.scalar.activation(
                        hid[:, m, :nb_tok], ph[:, :nb_tok], ACTF.Relu,
                        bias=b1[:, m : m + 1], scale=1.0,
                    )
                else:
                    nc.vector.tensor_scalar(
                        hid[:, m, :nb_tok], ph[:, :nb_tok], b1[:, m : m + 1], 0.0,
                        ALU.add, ALU.max,
                    )

            ofm = fpool.tile([128, HID // 128, 512], F32, tag="ofm")
            for m2 in range(HID // 128):
                po = ppool.tile([128, 512], F32, tag="mm", bufs=2)
                for k2 in range(H4 // 128):
                    nc.tensor.matmul(
                        out=po[:, :nb_tok],
                        lhsT=w2[:, k2, m2 * 128 : (m2 + 1) * 128],
                        rhs=hid[:, k2, :nb_tok],
                        start=(k2 == 0),
                        stop=(k2 == H4 // 128 - 1),
                    )
                nc.scalar.activation(
                    ofm[:, m2, :nb_tok], po[:, :nb_tok], ACTF.Identity,
                    bias=b2[:, m2 : m2 + 1], scale=1.0,
                )

            psmb = fpool.tile([128, 4, HID], F32, tag="xtok")
            nc.gpsimd.dma_gather(
                psmb[:, :kb, :],
                pose[:], psm16[:, wcols], nb_tok, nb_tok, HID,
            )
            fo = epool.tile([128, 4, HID], F32, tag="O", bufs=1)
            for ct in range(kb):
                eps_ps = ppool.tile([128, HID], F32, tag="smps", bufs=2)
                for m2 in range(HID // 128):
                    tp2 = ppool.tile([128, 128], F32, tag="tp")
                    nc.tensor.transpose(
                        out=tp2[:], in_=ofm[:, m2, ct * 128 : (ct + 1) * 128],
                        identity=ident[:],
                    )
                    nc.vector.tensor_copy(eps_ps[:, m2 * 128 : (m2 + 1) * 128], tp2[:])
                nc.tensor.matmul(
                    out=eps_ps[:, 0:512], lhsT=ident[:], rhs=psmb[:, ct, 0:512],
                    start=False, stop=True, skip_group_check=True,
                )
                nc.tensor.matmul(
                    out=eps_ps[:, 512:768], lhsT=ident[:], rhs=psmb[:, ct, 512:768],
                    start=False, stop=True, skip_group_check=True,
                )
                rs = spool.tile([128, 1], F32, tag="ln_rs", bufs=3)
                nbt = spool.tile([128, 1], F32, tag="ln_nb", bufs=3)
                ln_apply(eps_ps[:, 0:512], eps_ps[:, 512:768], fo[:, ct, :], rs, nbt)
                gb_apply(fo[:, ct, :])
            nc.gpsimd.dma_scatter_add(
                out[:],
                fo[:, :kb, :],
                dest16[:, wcols],
                nb_tok, nb_tok, HID,
            )
            joff += nb_tok

    nc.compile()
    return nc


_CACHE = {}


def _get_program(skip_gb):
    if skip_gb not in _CACHE:
        _CACHE[skip_gb] = build_program(skip_gb)
    return _CACHE[skip_gb]


def _host_constants():
    import ml_dtypes

    ident = np.eye(128, dtype=np.float32)
    identbf = ident.astype(ml_dtypes.bfloat16)
    lexclt = np.triu(np.ones((128, 128), np.float32), 1)
    ones_col = np.ones((128, 1), np.float32)
    ones_row = np.ones((1, 128), np.float32)
    iota_c = (np.arange(KJ)[None, :] * 128 + np.arange(128)[:, None]).astype(np.int32)
    return {
        "ident": ident, "identbf": identbf, "lexclt": lexclt,
        "ones_col": ones_col, "ones_row": ones_row, "iota_c": iota_c,
    }


def kernel(**inputs):
    fps = np.ascontiguousarray(np.asarray(inputs["SMILES_fps"], np.float32).reshape(B, S, FP))
    wtok = np.asarray(inputs["word_tokens_ref"]).astype(np.int32).reshape(B, S)
    vals = np.asarray(inputs["values_ref"], np.float32).reshape(B, S)
    ttyp = np.asarray(inputs["token_type_ids"]).astype(np.int32).reshape(B, S)
    posi = np.asarray(inputs["position_ids"]).astype(np.int32).reshape(B, S)

    ln_g = np.asarray(inputs["ln_g"], np.float32)
    ln_b = np.asarray(inputs["ln_b"], np.float32)
    skip_gb = bool(np.all(ln_g == 1.0) and np.all(ln_b == 0.0))
    nc = _get_program(skip_gb)

    shared = {
        "fc1_w": np.asarray(inputs["fc1_w"], np.float32),
        "fc1_b": np.asarray(inputs["fc1_b"], np.float32),
        "fc2_w": np.asarray(inputs["fc2_w"], np.float32),
        "fc2_b": np.asarray(inputs["fc2_b"], np.float32),
        "prop": np.asarray(inputs["prop_emb"], np.float32),
        "val_w": np.asarray(inputs["val_w"], np.float32),
        "val_b": np.asarray(inputs["val_b"], np.float32),
        "pose": np.asarray(inputs["pos_emb"], np.float32),
        "typee": np.asarray(inputs["type_emb"], np.float32),
        "ln_g": ln_g, "ln_b": ln_b,
    }
    shared.update(_host_constants())

    in_maps = []
    for c in range(N_CORES):
        b0, b1 = c * B_LOC, (c + 1) * B_LOC
        n_sm = int((ttyp[b0:b1] == 1).sum())
        assert n_sm <= CAP_S, f"smiles count {n_sm} exceeds capacity {CAP_S}"
        in_maps.append(
            dict(
                shared,
                fps=fps[b0:b1].reshape(N_TOK, FP),
                wtok=wtok[b0:b1].reshape(N_TOK),
                vals=vals[b0:b1].reshape(N_TOK),
                ttyp=ttyp[b0:b1].reshape(N_TOK),
                posi=posi[b0:b1].reshape(N_TOK),
            )
        )

    res = bass_utils.run_bass_kernel_spmd(nc, in_maps, core_ids=list(range(N_CORES)))
    full = np.concatenate(
        [res.results[c]["out"][:N_TOK].reshape(B_LOC, S, HID) for c in range(N_CORES)],
        axis=0,
    )
    return full



# revision 2
# speedup vs baseline: 1.6925x; 1.6925x over previous
"""Trainium2 Bass kernel for nn_MultiModalInputEmbeddings.

Data-parallel over batch: 8 cores x 8 batch rows = 4096 tokens/core.
Token slot convention is ROW-major: token t <-> (partition t//KJ, slot
t%KJ) so the dense output DMA writes contiguous per-partition blocks.

Host precomputes (numpy, cheap): the combined bf16 table
  ctab[0:1000]   = prop_emb + type_emb[0]     (word tokens)
  ctab[1000:1003]= type_emb[3:6]              (special tokens)
  ctab[1003]     = 0                          (smiles placeholder)
  ctab[1004]     = val_b + type_emb[2]        (value tokens)
plus bf16 pos table, a second pos table with fc2_b+type_emb[1] folded
(for the smiles branch), bf16 weights pre-tiled for the PE array, the
smiles compaction lists, and all gather index tiles in the wrapped
[16-partition] layout the dma_gather custom op wants.

Device per core (all bf16 data paths, fp32 PSUM/LN math):
  - Dense pass (all tokens), no TensorE: gather ctab+pos rows bf16,
    s = c + p + mval*val_w via DVE tensor_tensor + scalar_tensor_tensor,
    LayerNorm via bn_stats/bn_aggr + ACT sqrt + DVE apply, write bf16.
  - SMILES: one transpose-mode gather of bf16 fingerprints straight into
    feature-major layout; FFN1 feature-major (w1 stationary); FFN2
    token-major (hid chunk stationary, w2 moving) so no transposes are
    needed anywhere; pos rows (with b2+t1 folded) added during PSUM
    eviction; LN; indirect-scatter OVERWRITE of the dense rows.
"""

import sys

try:
    import concourse  # noqa: F401
except ImportError:  # pragma: no cover
    sys.path.insert(0, "/opt/trn_rl_repo")

import numpy as np

import concourse.bacc as bacc
import concourse.bass as bass  # noqa: F401
import concourse.mybir as mybir
import concourse.tile as tile
from concourse import bass_utils
from concourse.bass import IndirectOffsetOnAxis

F32 = mybir.dt.float32
BF16 = mybir.dt.bfloat16
I32 = mybir.dt.int32
I16 = mybir.dt.int16
ALU = mybir.AluOpType
ACTF = mybir.ActivationFunctionType

B, S, FP, HID = 64, 512, 768, 768
N_CORES = 8
B_LOC = B // N_CORES
N_TOK = B_LOC * S            # 4096 tokens/core
KJ = N_TOK // 128            # 32 slots per partition
NW = N_TOK // 16             # 256 wrapped-index columns
COL_VOCAB, MAX_POS = 1000, 512
H4 = 4 * FP
K1, M1 = FP // 128, H4 // 128       # 6, 24
CTAB_ROWS = 1008
ZROW = COL_VOCAB + 3
VROW = COL_VOCAB + 4
DUMP = N_TOK                 # output dump row for compaction padding
DG = 4                       # dense token-tiles per group
NG = KJ // DG                # 8 groups
EPS = 1e-12


def build_program(skip_gb: bool, cap: int):
    nch = cap // 128
    blocks = [(c0, min(c0 + 512, cap)) for c0 in range(0, cap, 512)]

    nc = bacc.Bacc(
        "TRN2",
        target_bir_lowering=False,
        debug=False,
        enable_asserts=False,
        num_devices=N_CORES,
        num_swdge_queues=4,
    )

    def din(name, shape, dt=F32):
        return nc.dram_tensor(name, shape, dt, kind="ExternalInput").ap()

    fps = din("fps", [N_TOK, FP], BF16)
    ctab = din("ctab", [CTAB_ROWS, HID], BF16)
    pose = din("pose", [MAX_POS, HID], BF16)
    pose2 = din("pose2", [MAX_POS, HID], BF16)
    w1d = din("w1", [128, K1, H4], BF16)
    w2d = din("w2", [128, M1, HID], BF16)
    b1d = din("b1", [128, M1])
    valw = din("valw", [1, HID], BF16)
    mvald = din("mval", [128, KJ])
    cidxd = din("cidx", [128, NW], I16)
    posd = din("posw", [128, NW], I16)
    sfwd = din("sfw", [128, cap // 16], I16)
    spwd = din("spw", [128, cap // 16], I16)
    sdestd = din("sdest", [128, nch], I32)
    ln_g = din("ln_g", [1, HID])
    ln_b = din("ln_b", [1, HID])

    out = nc.dram_tensor("out", [N_TOK + 128, HID], BF16, kind="ExternalOutput").ap()
    out_rm = out[:N_TOK, :].rearrange("(p j) f -> p j f", j=KJ)

    from contextlib import ExitStack

    with tile.TileContext(nc) as tc, ExitStack() as es:
        cpool = es.enter_context(tc.tile_pool(name="const", bufs=1))
        dpool = es.enter_context(tc.tile_pool(name="dense", bufs=2))
        fpool = es.enter_context(tc.tile_pool(name="ffn", bufs=1))
        ppool = es.enter_context(tc.tile_pool(name="psum", bufs=1, space="PSUM"))

        # ---- small constants (sync HWDGE ring) ----
        eps_t = cpool.tile([128, 1], F32)
        nc.vector.memset(eps_t[:], EPS)
        cidx_t = cpool.tile([128, NW], I16)
        nc.sync.dma_start(out=cidx_t[:], in_=cidxd[:])
        pos_t = cpool.tile([128, NW], I16)
        nc.sync.dma_start(out=pos_t[:], in_=posd[:])
        sfw_t = cpool.tile([128, cap // 16], I16)
        nc.sync.dma_start(out=sfw_t[:], in_=sfwd[:])
        spw_t = cpool.tile([128, cap // 16], I16)
        nc.sync.dma_start(out=spw_t[:], in_=spwd[:])
        sdest_t = cpool.tile([128, nch], I32)
        nc.sync.dma_start(out=sdest_t[:], in_=sdestd[:])
        mval_t = cpool.tile([128, KJ], F32)
        nc.sync.dma_start(out=mval_t[:], in_=mvald[:])
        b1_t = cpool.tile([128, M1], F32)
        nc.sync.dma_start(out=b1_t[:], in_=b1d[:])
        vwb = cpool.tile([128, HID], BF16)
        nc.sync.dma_start(out=vwb[:], in_=valw[0:1, :].to_broadcast([128, HID]))
        if not skip_gb:
            gb = cpool.tile([128, HID], F32)
            nc.scalar.dma_start(out=gb[:], in_=ln_g[0:1, :].to_broadcast([128, HID]))
            bb = cpool.tile([128, HID], F32)
            nc.scalar.dma_start(out=bb[:], in_=ln_b[0:1, :].to_broadcast([128, HID]))

        # ---- fingerprint transpose-gather straight into feature-major ----
        xfm = fpool.tile([128, K1, cap], BF16)
        nc.vector.memset(xfm[:], 0.0)
        nc.gpsimd.dma_gather(
            xfm[:], fps[:], sfw_t[:], cap, cap, FP, transpose=True, queue_num=0
        )
        # smiles pos rows (token-major; pose2 has fc2_b + type_emb[1] folded)
        pgt2 = fpool.tile([128, nch, HID], BF16)
        nc.gpsimd.dma_gather(
            pgt2[:], pose2[:], spw_t[:], cap, cap, HID, queue_num=3
        )

        # ---- weights, chunked so FFN1 m=0 can start early ----
        w1t = fpool.tile([128, K1, H4], BF16)
        for c in range(4):
            sl = slice(c * (H4 // 4), (c + 1) * (H4 // 4))
            nc.sync.dma_start(out=w1t[:, :, sl], in_=w1d[:, :, sl])
        w2t = fpool.tile([128, M1, HID], BF16)
        for c in range(4):
            sl = slice(c * (M1 // 4), (c + 1) * (M1 // 4))
            nc.sync.dma_start(out=w2t[:, sl, :], in_=w2d[:, sl, :])

        hid = fpool.tile([128, M1, cap], BF16)

        # ---- dense pass pieces ----
        def emit_dense_gather(g):
            cg = dpool.tile([128, DG, HID], BF16, tag="cg")
            nc.gpsimd.dma_gather(
                cg[:], ctab[:], cidx_t[:, g * DG * 8 : (g + 1) * DG * 8],
                DG * 128, DG * 128, HID, queue_num=1,
            )
            pg = dpool.tile([128, DG, HID], BF16, tag="pg")
            nc.gpsimd.dma_gather(
                pg[:], pose[:], pos_t[:, g * DG * 8 : (g + 1) * DG * 8],
                DG * 128, DG * 128, HID, queue_num=2,
            )
            return cg, pg

        def ln_tile(x_ap, o_ap, extra_dtype_hint=None):
            """LayerNorm stats+apply: o = (x - mean)/sqrt(var+eps) [*g +b]."""
            st = dpool.tile([128, 2, 6], F32, tag="ln_st", bufs=3)
            mv = dpool.tile([128, 2], F32, tag="ln_mv", bufs=3)
            nc.vector.bn_stats(st[:, 0, :], x_ap[:, 0:512])
            nc.vector.bn_stats(st[:, 1, :], x_ap[:, 512:768])
            nc.vector.bn_aggr(mv[:], st[:])
            std = dpool.tile([128, 1], F32, tag="ln_std", bufs=3)
            nc.scalar.activation(
                std[:], mv[:, 1:2], ACTF.Sqrt, bias=eps_t[:, 0:1], scale=1.0
            )
            rs = dpool.tile([128, 1], F32, tag="ln_rs", bufs=3)
            nc.vector.reciprocal(rs[:], std[:])
            nb = dpool.tile([128, 1], F32, tag="ln_nb", bufs=3)
            nc.vector.tensor_scalar(
                nb[:], mv[:, 0:1], rs[:, 0:1], -1.0, ALU.mult, ALU.mult
            )
            nc.vector.tensor_scalar(
                o_ap, x_ap, rs[:, 0:1], nb[:, 0:1], ALU.mult, ALU.add
            )
            if not skip_gb:
                nc.vector.tensor_tensor(out=o_ap, in0=o_ap, in1=gb[:], op=ALU.mult)
                nc.vector.tensor_tensor(out=o_ap, in0=o_ap, in1=bb[:], op=ALU.add)

        dense_state = {"next": 0, "cur": None}

        def emit_dense_group():
            g = dense_state["next"]
            if g >= NG:
                return
            dense_state["next"] = g + 1
            if g == 0:
                dense_state["cur"] = emit_dense_gather(0)
            cg, pg = dense_state["cur"]
            if g + 1 < NG:
                dense_state["cur"] = emit_dense_gather(g + 1)
            og = dpool.tile([128, DG, HID], BF16, tag="og")
            for jj in range(DG):
                j = g * DG + jj
                s = dpool.tile([128, HID], BF16, tag="s", bufs=3)
                nc.vector.tensor_tensor(
                    out=s[:], in0=cg[:, jj, :], in1=pg[:, jj, :], op=ALU.add
                )
                nc.vector.scalar_tensor_tensor(
                    out=s[:], in0=vwb[:], scalar=mval_t[:, j : j + 1], in1=s[:],
                    op0=ALU.mult, op1=ALU.add,
                )
                ln_tile(s[:], og[:, jj, :])
            nc.scalar.dma_start(
                out=out_rm[:, g * DG : (g + 1) * DG, :], in_=og[:]
            )

        # ---- FFN1: feature-major, w1 stationary; dense groups interleaved ----
        emit_dense_group()  # prime the gather pipeline before the matmul storm
        for m in range(M1):
            for c0, c1 in blocks:
                w = c1 - c0
                ph = ppool.tile([128, 512], F32, tag="ph", bufs=2)
                for k in range(K1):
                    nc.tensor.matmul(
                        out=ph[:, :w],
                        lhsT=w1t[:, k, m * 128 : (m + 1) * 128],
                        rhs=xfm[:, k, c0:c1],
                        start=(k == 0),
                        stop=(k == K1 - 1),
                    )
                nc.scalar.activation(
                    hid[:, m, c0:c1], ph[:, :w], ACTF.Relu,
                    bias=b1_t[:, m : m + 1], scale=1.0,
                )
            if m % 3 == 2:
                emit_dense_group()
        while dense_state["next"] < NG:
            emit_dense_group()

        # ---- FFN2: token-major (hid chunk stationary, w2 moving) ----
        for ct in range(nch):
            e_ps = ppool.tile([128, HID], F32, tag="eps", bufs=2)
            for k2 in range(M1):
                lh = hid[:, k2, ct * 128 : (ct + 1) * 128]
                nc.tensor.matmul(
                    out=e_ps[:, 0:512], lhsT=lh, rhs=w2t[:, k2, 0:512],
                    start=(k2 == 0), stop=(k2 == M1 - 1), skip_group_check=True,
                )
                nc.tensor.matmul(
                    out=e_ps[:, 512:768], lhsT=lh, rhs=w2t[:, k2, 512:768],
                    start=(k2 == 0), stop=(k2 == M1 - 1), skip_group_check=True,
                )
            fo = dpool.tile([128, HID], BF16, tag="fo", bufs=2)
            nc.vector.tensor_tensor(
                out=fo[:], in0=e_ps[:], in1=pgt2[:, ct, :], op=ALU.add
            )
            ln_tile(fo[:], fo[:])
            nc.gpsimd.indirect_dma_start(
                out=out[:],
                out_offset=IndirectOffsetOnAxis(ap=sdest_t[:, ct : ct + 1], axis=0),
                in_=fo[:],
                in_offset=None,
            )

    nc.compile()
    return nc


_CACHE = {}


def _get_program(skip_gb, cap):
    key = (skip_gb, cap)
    if key not in _CACHE:
        _CACHE[key] = build_program(skip_gb, cap)
    return _CACHE[key]


def _wrap_idx(lin):
    """[n] gather-order idx list -> [128, n//16] wrapped+replicated int16."""
    w16 = np.asarray(lin, np.int16).reshape(-1, 16).T  # [16, n/16]
    return np.tile(w16, (8, 1))


def kernel(**inputs):
    import ml_dtypes

    bf16 = ml_dtypes.bfloat16

    fps = np.asarray(inputs["SMILES_fps"], np.float32).reshape(B, S, FP)
    wtok = np.asarray(inputs["word_tokens_ref"]).astype(np.int64).reshape(B, S)
    vals = np.asarray(inputs["values_ref"], np.float32).reshape(B, S)
    ttyp = np.asarray(inputs["token_type_ids"]).astype(np.int64).reshape(B, S)
    posi = np.asarray(inputs["position_ids"]).astype(np.int64).reshape(B, S)

    prop = np.asarray(inputs["prop_emb"], np.float32)
    typee = np.asarray(inputs["type_emb"], np.float32)
    pos_emb = np.asarray(inputs["pos_emb"], np.float32)
    val_w = np.asarray(inputs["val_w"], np.float32)
    val_b = np.asarray(inputs["val_b"], np.float32)
    fc1_w = np.asarray(inputs["fc1_w"], np.float32)
    fc1_b = np.asarray(inputs["fc1_b"], np.float32)
    fc2_w = np.asarray(inputs["fc2_w"], np.float32)
    fc2_b = np.asarray(inputs["fc2_b"], np.float32)
    ln_g = np.asarray(inputs["ln_g"], np.float32)
    ln_b = np.asarray(inputs["ln_b"], np.float32)
    skip_gb = bool(np.all(ln_g == 1.0) and np.all(ln_b == 0.0))

    # ---- shared tables ----
    ctab = np.zeros((CTAB_ROWS, HID), np.float32)
    ctab[:COL_VOCAB] = prop + typee[0]
    ctab[COL_VOCAB : COL_VOCAB + 3] = typee[3:6]
    ctab[VROW] = val_b + typee[2]
    ctab_bf = ctab.astype(bf16)
    pose_bf = pos_emb.astype(bf16)
    pose2_bf = (pos_emb + fc2_b + typee[1]).astype(bf16)
    w1_bf = np.ascontiguousarray(
        fc1_w.reshape(K1, 128, H4).transpose(1, 0, 2)
    ).astype(bf16)
    w2_bf = np.ascontiguousarray(
        fc2_w.reshape(M1, 128, HID).transpose(1, 0, 2)
    ).astype(bf16)
    b1_t = np.ascontiguousarray(fc1_b.reshape(M1, 128).T)
    valw_bf = val_w.reshape(1, HID).astype(bf16)

    # dense gather order: slot i -> (p=i%128, j=i//128) holds token p*KJ+j0+j
    i_ar = np.arange(DG * 128)
    tok_of_slot = (i_ar % 128) * KJ + i_ar // 128  # within a group, j offset added

    # ---- per-core prep ----
    ttf = ttyp.reshape(N_CORES, N_TOK)
    wtf = wtok.reshape(N_CORES, N_TOK)
    vaf = vals.reshape(N_CORES, N_TOK)
    pof = posi.reshape(N_CORES, N_TOK)

    counts = [(ttf[c] == 1).sum() for c in range(N_CORES)]
    cap = max(768, int(np.ceil(max(counts) / 128)) * 128)
    nch = cap // 128

    nc = _get_program(skip_gb, cap)

    shared = {
        "ctab": ctab_bf, "pose": pose_bf, "pose2": pose2_bf,
        "w1": w1_bf, "w2": w2_bf, "b1": b1_t, "valw": valw_bf,
        "ln_g": ln_g.reshape(1, HID), "ln_b": ln_b.reshape(1, HID),
    }

    in_maps = []
    for c in range(N_CORES):
        tt, wt, va, po = ttf[c], wtf[c], vaf[c], pof[c]
        cidx_tok = np.full(N_TOK, ZROW, np.int64)
        cidx_tok[tt == 0] = wt[tt == 0]
        sp = tt >= 3
        cidx_tok[sp] = COL_VOCAB + tt[sp] - 3
        cidx_tok[tt == 2] = VROW

        cwrap = np.empty((16, NW), np.int16)
        pwrap = np.empty((16, NW), np.int16)
        for g in range(NG):
            toks = tok_of_slot + g * DG
            cw = _wrap_idx(cidx_tok[toks])[:16]
            pw = _wrap_idx(po[toks])[:16]
            cwrap[:, g * 32 : (g + 1) * 32] = cw
            pwrap[:, g * 32 : (g + 1) * 32] = pw
        cwrap = np.tile(cwrap, (8, 1))
        pwrap = np.tile(pwrap, (8, 1))

        mval_pj = np.where(tt == 2, va, 0.0).astype(np.float32).reshape(128, KJ)

        stok = np.nonzero(tt == 1)[0]
        n_sm = len(stok)
        assert n_sm <= cap, f"smiles count {n_sm} exceeds capacity {cap}"
        sfull = np.zeros(cap, np.int64)
        sfull[:n_sm] = stok
        sdest = np.full(nch * 128, DUMP, np.int64)
        sdest[:n_sm] = stok
        spos = np.zeros(cap, np.int64)
        spos[:n_sm] = po[stok]

        in_maps.append(
            dict(
                shared,
                fps=fps.reshape(N_CORES, N_TOK, FP)[c].astype(bf16),
                cidx=cwrap,
                posw=pwrap,
                mval=mval_pj,
                sfw=_wrap_idx(sfull),
                spw=_wrap_idx(spos),
                sdest=sdest.astype(np.int32).reshape(nch, 128).T.copy(),
            )
        )

    res = bass_utils.run_bass_kernel_spmd(nc, in_maps, core_ids=list(range(N_CORES)))
    full = np.concatenate(
        [
            np.asarray(res.results[c]["out"][:N_TOK], dtype=np.float32).reshape(
                B_LOC, S, HID
            )
            for c in range(N_CORES)
        ],
        axis=0,
    )
    return full
